# revision 26
# baseline (speedup 1.0000x reference)
"""Trainium2 Bass kernel for nn_LocalPointModel (kNN point-cloud network).

Sharding: 2 cores per cloud (B=4 x 2 halves = 8 cores). Each core computes
4096 query rows of its cloud. The per-core input cloud is ROTATED on the host
so that a core's queries are always rows 0:4096 (fully uniform SPMD kernel,
no dynamic addressing). kNN / gather / edge-MLP / heads are per-core local;
only the global ctx pooling (max+mean over the full cloud) needs a 2-core
AllReduce per cloud pair.

kNN: score[q,c] = <pq,pc> - |pc|^2/2  (= -d2/2 + const(q), order-preserving)
computed by a contract-dim-4 fp32 matmul; top-17 per row via hierarchical DVE
max8 (32 chunks of 256 -> survivor array M[128,256]) + 3 extraction rounds
(max8 + match_replace), then index recovery with full-row max_index. Row max
is always self (score diff = -d2/2 <= 0), so neighbors = extracted[1:17].

Edge MLP layer 1 is decomposed as  edge@W1 = X[q] + Y[c]  with
  Y = pf@Wn + pts@Wp + nrm@Wm     (gather source, per cloud point)
  X = pf@(Wc-Wn) - pts@Wp - nrm@Wm + b1   (per query)
so the gather feeds a single elementwise add instead of a 262x128 matmul
per edge. Neighbor gather = indirect_copy on columns of Y [H=128part, N].
"""

import numpy as np

import concourse.bass as bass
import concourse.bacc as bacc_mod
import concourse.mybir as mybir
from concourse.tile import TileContext
from concourse.bass_utils import run_bass_kernel_spmd

B, N, K, H, G, NCLS, PD = 4, 8192, 16, 128, 256, 32, 16
NQ = N // 2            # queries per core
QT = 128               # queries per tile
NT = NQ // QT          # 32 query tiles
FD = 512               # matmul moving-dim chunk
NF = N // FD           # 16
CH = 256               # top-k phase-1 chunk width
NCHK = N // CH         # 32 chunks
NEG = -3.0e38

f32 = mybir.dt.float32
bf16 = mybir.dt.bfloat16
u16 = mybir.dt.uint16
i16 = mybir.dt.int16
AF = mybir.ActivationFunctionType
ALU = mybir.AluOpType

# (name, shape) of every weight input, in reference order
WEIGHT_SPECS = [
    ("W_pe1", (6, H)), ("b_pe1", (H,)), ("W_pe2", (H, H)), ("b_pe2", (H,)),
    ("W_lm1", (2 * H + 6, H)), ("b_lm1", (H,)), ("W_lm2", (H, H)), ("b_lm2", (H,)),
    ("W_cp", (2 * H, G)), ("b_cp", (G,)), ("W_gp", (2 * G, G)), ("b_gp", (G,)),
    ("W_cl1", (2 * H + G, H)), ("b_cl1", (H,)), ("W_cl2", (H, NCLS)), ("b_cl2", (NCLS,)),
    ("W_ph1", (2 * H + G, H)), ("b_ph1", (H,)), ("W_ph2", (H, PD)), ("b_ph2", (PD,)),
    ("W_bh1", (2 * H + G, H)), ("b_bh1", (H,)), ("W_bh2", (H, 1)), ("b_bh2", (1,)),
]


def build_program():
    nc = bacc_mod.Bacc(num_devices=8)

    cpts = nc.declare_dram_parameter("cpts", [N, 3], f32, isOutput=False)
    cnrm = nc.declare_dram_parameter("cnrm", [N, 3], f32, isOutput=False)
    cmask_d = nc.declare_dram_parameter("cmask", [1, 2 * B], f32, isOutput=False)
    wd = {
        name: nc.declare_dram_parameter(name, list(shape), f32, isOutput=False)
        for name, shape in WEIGHT_SPECS
    }
    out_d = nc.declare_dram_parameter("out", [NQ, NCLS + PD + 1], f32, isOutput=True)

    # collective scratch: per-cloud slots, all-8 AllReduce (pads are neutral)
    cc_max_in = nc.dram_tensor("cc_max_in", [H, 2 * B], f32)
    cc_max_out = nc.dram_tensor("cc_max_out", [H, 2 * B], f32, addr_space="Shared")
    cc_sum_in = nc.dram_tensor("cc_sum_in", [H, 2 * B], f32)
    cc_sum_out = nc.dram_tensor("cc_sum_out", [H, 2 * B], f32, addr_space="Shared")
    GROUPS = [[0, 1, 2, 3, 4, 5, 6, 7]]

    with TileContext(nc) as tc:
        with tc.tile_pool(name="persist", bufs=1) as pp:
            # ---------------- persistent tiles ----------------
            cw = pp.tile([4, N], f32, tag="cw")        # score rhs: [px,py,pz,p2/2]
            pf = pp.tile([H, N], bf16, tag="pf")       # point features, full cloud
            ysb = pp.tile([H, N], f32, tag="ysb")      # gather source Y
            xq = pp.tile([H, NQ], bf16, tag="xq")      # per-query X (+b_lm1)
            lf = pp.tile([H, NQ], bf16, tag="lf")      # edge-pooled features
            gmax = pp.tile([H, 2], f32, tag="gmax")
            gsum = pp.tile([H, 2], f32, tag="gsum")
            gstat = pp.tile([H, 4], bf16, tag="gstat")
            gf_bc = pp.tile([H, 2 * QT], bf16, tag="gfbc")  # gf broadcast (2 G-blocks)

            # ---------------- weights / biases ----------------
            def load_w_bf(name, rows, row0=0, cols=None, col0=0, scale=None):
                cols = cols if cols is not None else wd[name].shape[1]
                t_f = pp.tile([rows, cols], f32, tag=f"{name}_{row0}_{col0}_{scale}_f")
                nc.gpsimd.dma_start(t_f[:, :], wd[name][row0:row0 + rows, col0:col0 + cols])
                t_b = pp.tile([rows, cols], bf16, tag=f"{name}_{row0}_{col0}_{scale}_b")
                if scale is None:
                    nc.vector.tensor_copy(t_b[:, :], t_f[:, :])
                else:
                    nc.vector.tensor_scalar_mul(t_b[:, :], t_f[:, :], scale)
                return t_f, t_b

            def load_bias(name, rows, row0=0):
                t = pp.tile([rows, 1], f32, tag=f"{name}_{row0}")
                nc.gpsimd.dma_start(t[:, :], wd[name][row0:row0 + rows].rearrange("o -> o ()"))
                return t

            _, wpe1 = load_w_bf("W_pe1", 6)
            _, wpe2 = load_w_bf("W_pe2", H)
            wl1a_f, _ = load_w_bf("W_lm1", H, 0)
            wl1b_f, wl1b = load_w_bf("W_lm1", H, H)
            _, wl1pm = load_w_bf("W_lm1", 6, 2 * H)
            _, wl1pm_n = load_w_bf("W_lm1", 6, 2 * H, scale=-1.0)
            wdiff = pp.tile([H, H], bf16, tag="wdiff")
            nc.vector.tensor_sub(wdiff[:, :], wl1a_f[:, :], wl1b_f[:, :])
            _, wlm2 = load_w_bf("W_lm2", H)
            wcp = [[load_w_bf("W_cp", H, r * H, H, gb * H)[1] for r in range(2)]
                   for gb in range(2)]
            wgp = [[load_w_bf("W_gp", H, c * H, H, gb * H)[1] for c in range(4)]
                   for gb in range(2)]
            wh1 = {h: [load_w_bf(f"W_{h}1", H, c * H)[1] for c in range(4)]
                   for h in ("cl", "ph", "bh")}
            wh2 = {h: load_w_bf(f"W_{h}2", H)[1] for h in ("cl", "ph", "bh")}

            b_pe1 = load_bias("b_pe1", H)
            b_pe2 = load_bias("b_pe2", H)
            b_lm1 = load_bias("b_lm1", H)
            b_lm2 = load_bias("b_lm2", H)
            b_cp = [load_bias("b_cp", H, gb * H) for gb in range(2)]
            b_gp = [load_bias("b_gp", H, gb * H) for gb in range(2)]
            b_h1 = {h: load_bias(f"b_{h}1", H) for h in ("cl", "ph", "bh")}
            # head-2 biases: added via rank-1 (ones x bias) matmul accumulate
            b_h2row = {}
            for h, od in (("cl", NCLS), ("ph", PD), ("bh", 1)):
                row = pp.tile([1, od], f32, tag=f"b_{h}2_row")
                nc.gpsimd.dma_start(row[:, :], wd[f"b_{h}2"][:].rearrange("o -> () o"))
                rowb = pp.tile([1, od], bf16, tag=f"b_{h}2_rowb")
                nc.vector.tensor_copy(rowb[:, :], row[:, :])
                b_h2row[h] = rowb
            ones1 = pp.tile([1, QT], bf16, tag="ones1")
            nc.vector.memset(ones1[:, :], 1.0)

            ones3 = pp.tile([3, 1], f32, tag="ones3")
            nc.vector.memset(ones3[:, :], 1.0)

            # ---------------- prep: xT, cw, pf, Y, X ----------------
            with tc.tile_pool(name="prep", bufs=2) as prep, \
                 tc.tile_pool(name="prep_ps", bufs=3, space="PSUM") as prep_ps, \
                 tc.tile_pool(name="prep1", bufs=1) as prep1:
                xt6 = prep1.tile([6, N], f32, tag="xt6")
                xb6 = prep1.tile([6, N], bf16, tag="xb6")
                nc.sync.dma_start(xt6[0:3, :], cpts[:, :].rearrange("n c -> c n"))
                nc.sync.dma_start(xt6[3:6, :], cnrm[:, :].rearrange("n c -> c n"))
                nc.vector.tensor_copy(xb6[:, :], xt6[:, :])
                nc.vector.tensor_copy(cw[0:3, :], xt6[0:3, :])
                p2row = prep1.tile([1, N], f32, tag="p2row")
                for c in range(NF):
                    sl = slice(c * FD, (c + 1) * FD)
                    sq = prep.tile([3, FD], f32, tag="sq")
                    nc.vector.tensor_mul(sq[:, :], xt6[0:3, sl], xt6[0:3, sl])
                    ps_p2 = prep_ps.tile([1, FD], f32, tag="pps")
                    nc.tensor.matmul(ps_p2[:, :], ones3[:, :], sq[:, :],
                                     start=True, stop=True)
                    nc.scalar.mul(p2row[0:1, sl], ps_p2[:, :], 0.5)
                nc.sync.dma_start(cw[3:4, :], p2row[:, :])
                for c in range(NF):
                    sl = slice(c * FD, (c + 1) * FD)
                    ps_a = prep_ps.tile([H, FD], f32, tag="pps")
                    nc.tensor.matmul(ps_a[:, :], wpe1[:, :], xb6[:, sl],
                                     start=True, stop=True)
                    hch = prep.tile([H, FD], bf16, tag="hch")
                    nc.scalar.activation(hch[:, :], ps_a[:, :], AF.Relu,
                                         bias=b_pe1[:, 0:1], scale=1.0)
                    ps_b = prep_ps.tile([H, FD], f32, tag="pps")
                    nc.tensor.matmul(ps_b[:, :], wpe2[:, :], hch[:, :],
                                     start=True, stop=True)
                    nc.scalar.activation(pf[:, sl], ps_b[:, :], AF.Relu,
                                         bias=b_pe2[:, 0:1], scale=1.0)
                for c in range(NF):
                    sl = slice(c * FD, (c + 1) * FD)
                    ps_y = prep_ps.tile([H, FD], f32, tag="pps")
                    nc.tensor.matmul(ps_y[:, :], wl1b[:, :], pf[:, sl],
                                     start=True, stop=False)
                    nc.tensor.matmul(ps_y[:, :], wl1pm[:, :], xb6[:, sl],
                                     start=False, stop=True)
                    nc.scalar.copy(ysb[:, sl], ps_y[:, :])
                for c in range(NQ // FD):
                    sl = slice(c * FD, (c + 1) * FD)
                    ps_x = prep_ps.tile([H, FD], f32, tag="pps")
                    nc.tensor.matmul(ps_x[:, :], wdiff[:, :], pf[:, sl],
                                     start=True, stop=False)
                    nc.tensor.matmul(ps_x[:, :], wl1pm_n[:, :], xb6[:, sl],
                                     start=False, stop=True)
                    nc.scalar.activation(xq[:, sl], ps_x[:, :], AF.Identity,
                                         bias=b_lm1[:, 0:1], scale=1.0)

            nc.vector.memset(gmax[:, :], NEG)
            nc.vector.memset(gsum[:, :], 0.0)

            # ---------------- main loop A: kNN + edge MLP + ctx stats ----------
            with tc.tile_pool(name="scores_p", bufs=2) as scores_p, \
                 tc.tile_pool(name="loopa", bufs=2) as la, \
                 tc.tile_pool(name="loopa1", bufs=1) as la1, \
                 tc.tile_pool(name="loopa4", bufs=2) as la4, \
                 tc.tile_pool(name="ps_sc", bufs=4, space="PSUM") as ps_sc, \
                 tc.tile_pool(name="ps_h2", bufs=2, space="PSUM") as ps_h2, \
                 tc.tile_pool(name="ps_cx", bufs=2, space="PSUM") as ps_cx:
                for t in range(NT):
                    tq = slice(t * QT, (t + 1) * QT)
                    # --- scores = <pq,pc> - p2c/2 ---
                    qlt = la.tile([4, QT], f32, tag="qlt")
                    nc.vector.memset(qlt[:, :], -1.0)
                    nc.vector.tensor_copy(qlt[0:3, :], cw[0:3, tq])
                    scores = scores_p.tile([QT, N], f32, tag="scores")
                    for c in range(NF):
                        sl = slice(c * FD, (c + 1) * FD)
                        ps = ps_sc.tile([QT, FD], f32, tag="ps_s")
                        nc.tensor.matmul(ps[:, :], qlt[:, :], cw[:, sl],
                                         start=True, stop=True)
                        nc.scalar.copy(scores[:, sl], ps[:, :])
                    # --- top-17: chunk max8 -> 3 extraction rounds ---
                    m_t = la1.tile([QT, NCHK * 8], f32, tag="m_t")
                    for c in range(NCHK):
                        nc.vector.max(m_t[:, c * 8:(c + 1) * 8],
                                      scores[:, c * CH:(c + 1) * CH])
                    tt = la.tile([QT, 24], f32, tag="tt")
                    nc.vector.max(tt[:, 0:8], m_t[:, :])
                    nc.vector.match_replace(m_t[:, :], tt[:, 0:8], m_t[:, :], NEG)
                    nc.vector.max(tt[:, 8:16], m_t[:, :])
                    nc.vector.match_replace(m_t[:, :], tt[:, 8:16], m_t[:, :], NEG)
                    nc.vector.max(tt[:, 16:24], m_t[:, :])
                    it_ = la.tile([QT, 24], u16, tag="it_")
                    for r in range(3):
                        nc.vector.max_index(it_[:, r * 8:(r + 1) * 8],
                                            tt[:, r * 8:(r + 1) * 8], scores[:, :])
                    # --- wrap indices for the column gather ---
                    ipad = la.tile([QT, 128], u16, tag="ipad")
                    nc.vector.tensor_copy(ipad[:, 0:16], it_[:, 1:17])
                    nc.vector.tensor_copy(ipad[:, 16:32], it_[:, 1:17])
                    itt = la.tile([128, QT], u16, tag="itt")
                    nc.sync.dma_start_transpose(itt[:, :], ipad[:, :])
                    idxs = la4.tile([H, QT], i16, tag="idxs")
                    for g in range(4):
                        nc.gpsimd.tensor_copy(idxs[g * 32:(g + 1) * 32, :],
                                              itt[0:32, :])
                    yg = la1.tile([H, QT * K], f32, tag="yg")
                    nc.gpsimd.ap_gather(
                        yg[:, :].rearrange("p m -> p m ()"),
                        ysb[:, :].rearrange("p n -> p n ()"),
                        idxs[:, :], channels=H, num_elems=N, d=1,
                        num_idxs=QT * K)
                    # --- h1 = relu(Y[c] + X[q]) ; h2 ; max over K ---
                    h1 = la.tile([H, QT * K], bf16, tag="h1")
                    nc.vector.tensor_tensor(
                        h1[:, :].rearrange("p (q j) -> p q j", j=K),
                        yg[:, :].rearrange("p (q j) -> p q j", j=K),
                        xq[:, tq].rearrange("p q -> p q ()").broadcast_to([H, QT, K]),
                        op=ALU.add)
                    nc.scalar.activation(h1[:, :], h1[:, :], AF.Relu, bias=0.0,
                                         scale=1.0)
                    lfr = la.tile([H, QT], f32, tag="lfr")
                    for c in range(4):
                        ps2 = ps_h2.tile([H, FD], f32, tag="ps2")
                        nc.tensor.matmul(ps2[:, :], wlm2[:, :],
                                         h1[:, c * FD:(c + 1) * FD],
                                         start=True, stop=True)
                        nc.vector.reduce_max(
                            lfr[:, c * 32:(c + 1) * 32],
                            ps2[:, :].rearrange("p (q j) -> p q j", j=K),
                            axis=mybir.AxisListType.X)
                    nc.scalar.activation(lf[:, tq], lfr[:, :], AF.Relu,
                                         bias=b_lm2[:, 0:1], scale=1.0)
                    # --- ctx + global pooling stats ---
                    for gb in range(2):
                        psc = ps_cx.tile([H, QT], f32, tag="psc")
                        nc.tensor.matmul(psc[:, :], wcp[gb][0][:, :], pf[:, tq],
                                         start=True, stop=False)
                        nc.tensor.matmul(psc[:, :], wcp[gb][1][:, :], lf[:, tq],
                                         start=False, stop=True)
                        cxs = la.tile([H, QT], f32, tag="cxs")
                        nc.scalar.activation(cxs[:, :], psc[:, :], AF.Relu,
                                             bias=b_cp[gb][:, 0:1], scale=1.0)
                        red = la.tile([H, 2], f32, tag="red")
                        nc.vector.reduce_max(red[:, 0:1], cxs[:, :],
                                             axis=mybir.AxisListType.X)
                        nc.vector.reduce_sum(red[:, 1:2], cxs[:, :],
                                             axis=mybir.AxisListType.X)
                        nc.vector.tensor_max(gmax[:, gb:gb + 1], gmax[:, gb:gb + 1],
                                             red[:, 0:1])
                        nc.vector.tensor_add(gsum[:, gb:gb + 1], gsum[:, gb:gb + 1],
                                             red[:, 1:2])

            # ---------------- global pooling collective + gf ----------------
            # All-8 AllReduce over per-cloud slots (cols 2b:2b+2); each core
            # contributes only its own cloud's slot (neutral pads elsewhere).
            cmask_row = pp.tile([1, 2 * B], f32, tag="cmask_row")
            nc.sync.dma_start(cmask_row[:, :], cmask_d[:, :])
            cmask_rowb = pp.tile([1, 2 * B], bf16, tag="cmask_rowb")
            nc.vector.tensor_copy(cmask_rowb[:, :], cmask_row[:, :])
            cmask_bc = pp.tile([H, 2 * B], f32, tag="cmask_bc")
            with tc.tile_pool(name="ps_cm", bufs=1, space="PSUM") as ps_cm:
                ps_c = ps_cm.tile([H, 2 * B], f32, tag="ps_c")
                nc.tensor.matmul(ps_c[:, :], ones1[:, :], cmask_rowb[:, :],
                                 start=True, stop=True)
                nc.scalar.copy(cmask_bc[:, :], ps_c[:, :])
            cpad = pp.tile([H, 2 * B], f32, tag="cpad")
            nc.vector.tensor_scalar(cpad[:, :], cmask_bc[:, :], -NEG, NEG,
                                    op0=ALU.mult, op1=ALU.add)
            gin = pp.tile([H, 2 * B], f32, tag="gin")

            def rep(ap):  # [H,2] -> [H,B,2] broadcast view
                return ap.rearrange("p c -> p () c").broadcast_to([H, B, 2])

            gin_v = gin[:, :].rearrange("p (b c) -> p b c", c=2)
            cm_v = cmask_bc[:, :].rearrange("p (b c) -> p b c", c=2)
            nc.vector.tensor_mul(gin_v, rep(gmax[:, :]), cm_v)
            nc.vector.tensor_add(gin[:, :], gin[:, :], cpad[:, :])
            nc.sync.dma_start(cc_max_in[:, :], gin[:, :])
            nc.gpsimd.collective_compute(
                "AllReduce", ALU.max, replica_groups=GROUPS,
                ins=[cc_max_in[:, :]], outs=[cc_max_out[:, :]])
            nc.vector.tensor_mul(gin_v, rep(gsum[:, :]), cm_v)
            nc.sync.dma_start(cc_sum_in[:, :], gin[:, :])
            nc.gpsimd.collective_compute(
                "AllReduce", ALU.add, replica_groups=GROUPS,
                ins=[cc_sum_in[:, :]], outs=[cc_sum_out[:, :]])
            gmaxr = pp.tile([H, 2 * B], f32, tag="gmaxr")
            gsumr = pp.tile([H, 2 * B], f32, tag="gsumr")
            nc.sync.dma_start(gmaxr[:, :], cc_max_out[:, :])
            nc.sync.dma_start(gsumr[:, :], cc_sum_out[:, :])
            # mask out other clouds (ctx >= 0 so 0-pads are neutral for max too)
            nc.vector.tensor_mul(gmaxr[:, :], gmaxr[:, :], cmask_bc[:, :])
            nc.vector.tensor_mul(gsumr[:, :], gsumr[:, :], cmask_bc[:, :])
            own = pp.tile([H, 4], f32, tag="own")
            nc.vector.reduce_max(
                own[:, 0:2], gmaxr[:, :].rearrange("p (b c) -> p c b", c=2),
                axis=mybir.AxisListType.X)
            nc.vector.reduce_sum(
                own[:, 2:4], gsumr[:, :].rearrange("p (b c) -> p c b", c=2),
                axis=mybir.AxisListType.X)
            nc.vector.tensor_copy(gstat[:, 0:2], own[:, 0:2])
            nc.vector.tensor_scalar_mul(gstat[:, 2:4], own[:, 2:4], 1.0 / N)

            with tc.tile_pool(name="ps_gf", bufs=2, space="PSUM") as ps_gf:
                for gb in range(2):
                    psg = ps_gf.tile([H, 1], f32, tag="psg")
                    for c in range(4):
                        nc.tensor.matmul(psg[:, :], wgp[gb][c][:, :],
                                         gstat[:, c:c + 1],
                                         start=(c == 0), stop=(c == 3))
                    gfv = pp.tile([H, 1], bf16, tag=f"gfv{gb}")
                    nc.scalar.activation(gfv[:, :], psg[:, :], AF.Relu,
                                         bias=b_gp[gb][:, 0:1], scale=1.0)
                    nc.vector.tensor_copy(
                        gf_bc[:, gb * QT:(gb + 1) * QT],
                        gfv[:, 0:1].broadcast_to([H, QT]))

            # ---------------- loop B: heads ----------------
            with tc.tile_pool(name="loopb", bufs=3) as lb, \
                 tc.tile_pool(name="ps_b1", bufs=3, space="PSUM") as ps_b1, \
                 tc.tile_pool(name="ps_b2", bufs=3, space="PSUM") as ps_b2:
                for t in range(NT):
                    tq = slice(t * QT, (t + 1) * QT)
                    osb = lb.tile([QT, NCLS + PD + 1], f32, tag="osb")
                    for h, od, off in (("cl", NCLS, 0), ("ph", PD, NCLS),
                                       ("bh", 1, NCLS + PD)):
                        psh = ps_b1.tile([H, QT], f32, tag="psh")
                        nc.tensor.matmul(psh[:, :], wh1[h][0][:, :], pf[:, tq],
                                         start=True, stop=False)
                        nc.tensor.matmul(psh[:, :], wh1[h][1][:, :], lf[:, tq],
                                         start=False, stop=False)
                        nc.tensor.matmul(psh[:, :], wh1[h][2][:, :],
                                         gf_bc[:, 0:QT], start=False, stop=False)
                        nc.tensor.matmul(psh[:, :], wh1[h][3][:, :],
                                         gf_bc[:, QT:2 * QT], start=False, stop=True)
                        hx = lb.tile([H, QT], bf16, tag=f"hx{h}")
                        nc.scalar.activation(hx[:, :], psh[:, :], AF.Relu,
                                             bias=b_h1[h][:, 0:1], scale=1.0)
                        ps2h = ps_b2.tile([QT, od], f32, tag="ps2h")
                        nc.tensor.matmul(ps2h[:, :], hx[:, :], wh2[h][:, 0:od],
                                         start=True, stop=False)
                        nc.tensor.matmul(ps2h[:, :], ones1[:, :],
                                         b_h2row[h][:, :], start=False, stop=True)
                        nc.scalar.copy(osb[:, off:off + od], ps2h[:, :])
                    nc.sync.dma_start(out_d[t * QT:(t + 1) * QT, :], osb[:, :])

    nc.finalize()
    _legalize_waits(nc)
    return nc


DMA_WAIT_LIMIT_TYPES = ("InstDMACopy",)


def _legalize_waits(nc):
    """TPB instructions hold 1 sync wait (DMA descriptors 2); move overflow
    onto same-engine single-wait Drains inserted just before (engine
    programs execute in-order, so the waits still gate the instruction)."""
    for func in nc.m.functions:
        for block in func.blocks:
            out = []
            for ins in block.instructions:
                si = ins.sync_info
                limit = 1
                if si is not None and len(si.on_wait) > limit:
                    waits = list(si.on_wait)
                    keep = waits[-limit:]
                    for i, w in enumerate(waits[:-limit]):
                        d = mybir.InstDrain(name=f"{ins.name}-w{i}", ins=[],
                                            outs=[], bass_is_fusable=False)
                        d.engine = ins.engine
                        d.sync_info = mybir.SyncInfo(on_wait=[w], on_update=[])
                        out.append(d)
                    ins.sync_info = mybir.SyncInfo(
                        on_wait=keep, on_update=list(si.on_update))
                out.append(ins)
            block.instructions = out


_CACHED = {}


def _get_program():
    if "nc" not in _CACHED:
        _CACHED["nc"] = build_program()
    return _CACHED["nc"]


def run_cores(inputs, trace=False):
    """Build per-core input maps, run on 8 cores, return per-core 'out' arrays."""
    points = np.asarray(inputs["points"], np.float32)
    normals = np.asarray(inputs["normals"], np.float32)
    in_maps = []
    for core in range(8):
        b, half = core // 2, core % 2
        shift = -half * NQ
        cmask = np.zeros((1, 2 * B), np.float32)
        cmask[0, 2 * b:2 * b + 2] = 1.0
        m = {
            "cpts": np.ascontiguousarray(np.roll(points[b], shift, axis=0)),
            "cnrm": np.ascontiguousarray(np.roll(normals[b], shift, axis=0)),
            "cmask": cmask,
        }
        for name, _ in WEIGHT_SPECS:
            m[name] = np.ascontiguousarray(np.asarray(inputs[name], np.float32))
        in_maps.append(m)
    nc = _get_program()
    res = run_bass_kernel_spmd(nc, in_maps, core_ids=list(range(8)), trace=trace)
    return res


def kernel(**inputs):
    assert int(inputs["k"]) == K
    res = run_cores(inputs, trace=False)
    outs = [res.results[c]["out"] for c in range(8)]
    logits = np.zeros((B, N, NCLS), np.float32)
    param = np.zeros((B, N, PD), np.float32)
    boundary = np.zeros((B, N), np.float32)
    for core in range(8):
        b, half = core // 2, core % 2
        o = outs[core].reshape(NQ, NCLS + PD + 1)
        rows = slice(half * NQ, (half + 1) * NQ)
        logits[b, rows] = o[:, :NCLS]
        param[b, rows] = o[:, NCLS:NCLS + PD]
        boundary[b, rows] = o[:, NCLS + PD]
    return logits, param, boundary


def build_trivial_program():
    """Same I/O signature, near-zero compute: for overhead-differencing."""
    nc = bacc_mod.Bacc(num_devices=8)
    nc.declare_dram_parameter("cpts", [N, 3], f32, isOutput=False)
    nc.declare_dram_parameter("cnrm", [N, 3], f32, isOutput=False)
    cmask_d = nc.declare_dram_parameter("cmask", [1, 2 * B], f32, isOutput=False)
    for name, shape in WEIGHT_SPECS:
        nc.declare_dram_parameter(name, list(shape), f32, isOutput=False)
    out_d = nc.declare_dram_parameter("out", [NQ, NCLS + PD + 1], f32,
                                      isOutput=True)
    with TileContext(nc) as tc:
        with tc.tile_pool(name="t", bufs=1) as tp:
            t = tp.tile([1, 2 * B], f32, tag="t")
            nc.sync.dma_start(t[:, :], cmask_d[:, :])
            nc.sync.dma_start(out_d[0:1, 0:2 * B], t[:, :])
    nc.finalize()
    _legalize_waits(nc)
    return nc


def run_cores_nc(nc, inputs, reps=1):
    import time as _t
    points = np.asarray(inputs["points"], np.float32)
    normals = np.asarray(inputs["normals"], np.float32)
    in_maps = []
    for core in range(8):
        b, half = core // 2, core % 2
        cmask = np.zeros((1, 2 * B), np.float32)
        cmask[0, 2 * b:2 * b + 2] = 1.0
        m = {
            "cpts": np.ascontiguousarray(np.roll(points[b], -half * NQ, axis=0)),
            "cnrm": np.ascontiguousarray(np.roll(normals[b], -half * NQ, axis=0)),
            "cmask": cmask,
        }
        for name, _ in WEIGHT_SPECS:
            m[name] = np.ascontiguousarray(np.asarray(inputs[name], np.float32))
        in_maps.append(m)
    times = []
    for _ in range(reps):
        t0 = _t.time()
        res = run_bass_kernel_spmd(nc, in_maps, core_ids=list(range(8)))
        times.append(_t.time() - t0)
    return res, times


# revision 28
# speedup vs baseline: 1.1029x; 1.1029x over previous
"""Trainium2 Bass kernel for nn_LocalPointModel (kNN point-cloud network).

Sharding: 2 cores per cloud (B=4 x 2 halves = 8 cores). Each core computes
4096 query rows of its cloud. The per-core input cloud is ROTATED on the host
so that a core's queries are always rows 0:4096 (fully uniform SPMD kernel,
no dynamic addressing). kNN / gather / edge-MLP / heads are per-core local;
only the global ctx pooling (max+mean over the full cloud) needs a 2-core
AllReduce per cloud pair.

kNN: score[q,c] = <pq,pc> - |pc|^2/2  (= -d2/2 + const(q), order-preserving)
computed by a contract-dim-4 fp32 matmul; top-17 per row via hierarchical DVE
max8 (32 chunks of 256 -> survivor array M[128,256]) + 3 extraction rounds
(max8 + match_replace), then index recovery with full-row max_index. Row max
is always self (score diff = -d2/2 <= 0), so neighbors = extracted[1:17].

Edge MLP layer 1 is decomposed as  edge@W1 = X[q] + Y[c]  with
  Y = pf@Wn + pts@Wp + nrm@Wm     (gather source, per cloud point)
  X = pf@(Wc-Wn) - pts@Wp - nrm@Wm + b1   (per query)
so the gather feeds a single elementwise add instead of a 262x128 matmul
per edge. Neighbor gather = indirect_copy on columns of Y [H=128part, N].
"""

import os
import numpy as np

import concourse.bass as bass
import concourse.bacc as bacc_mod
import concourse.mybir as mybir
from concourse.tile import TileContext
from concourse.bass_utils import run_bass_kernel_spmd

B, N, K, H, G, NCLS, PD = 4, 8192, 16, 128, 256, 32, 16
NQ = N // 2            # queries per core
QT = 128               # queries per tile
NT = NQ // QT          # 32 query tiles
FD = 512               # matmul moving-dim chunk
NF = N // FD           # 16
CH = 512               # top-k phase-1 chunk width
NCHK = N // CH         # 32 chunks
NEG = -3.0e38

f32 = mybir.dt.float32
bf16 = mybir.dt.bfloat16
u16 = mybir.dt.uint16
i16 = mybir.dt.int16
AF = mybir.ActivationFunctionType
ALU = mybir.AluOpType

# (name, shape) of every weight input, in reference order
WEIGHT_SPECS = [
    ("W_pe1", (6, H)), ("b_pe1", (H,)), ("W_pe2", (H, H)), ("b_pe2", (H,)),
    ("W_lm1", (2 * H + 6, H)), ("b_lm1", (H,)), ("W_lm2", (H, H)), ("b_lm2", (H,)),
    ("W_cp", (2 * H, G)), ("b_cp", (G,)), ("W_gp", (2 * G, G)), ("b_gp", (G,)),
    ("W_cl1", (2 * H + G, H)), ("b_cl1", (H,)), ("W_cl2", (H, NCLS)), ("b_cl2", (NCLS,)),
    ("W_ph1", (2 * H + G, H)), ("b_ph1", (H,)), ("W_ph2", (H, PD)), ("b_ph2", (PD,)),
    ("W_bh1", (2 * H + G, H)), ("b_bh1", (H,)), ("W_bh2", (H, 1)), ("b_bh2", (1,)),
]


def build_program():
    nc = bacc_mod.Bacc(num_devices=8)

    cpts = nc.declare_dram_parameter("cpts", [N, 3], f32, isOutput=False)
    cnrm = nc.declare_dram_parameter("cnrm", [N, 3], f32, isOutput=False)
    cmask_d = nc.declare_dram_parameter("cmask", [1, 2 * B], f32, isOutput=False)
    wd = {
        name: nc.declare_dram_parameter(name, list(shape), f32, isOutput=False)
        for name, shape in WEIGHT_SPECS
    }
    out_d = nc.declare_dram_parameter("out", [NQ, NCLS + PD + 1], f32, isOutput=True)

    # collective scratch: per-cloud slots, all-8 AllReduce (pads are neutral)
    cc_max_in = nc.dram_tensor("cc_max_in", [H, 2 * B], f32)
    cc_max_out = nc.dram_tensor("cc_max_out", [H, 2 * B], f32, addr_space="Shared")
    cc_sum_in = nc.dram_tensor("cc_sum_in", [H, 2 * B], f32)
    cc_sum_out = nc.dram_tensor("cc_sum_out", [H, 2 * B], f32, addr_space="Shared")
    GROUPS = [[0, 1, 2, 3, 4, 5, 6, 7]]

    with TileContext(nc) as tc:
        with tc.tile_pool(name="persist", bufs=1) as pp:
            # ---------------- persistent tiles ----------------
            cw = pp.tile([4, N], f32, tag="cw")        # score rhs: [px,py,pz,p2/2]
            pf = pp.tile([H, N], bf16, tag="pf")       # point features, full cloud
            ysb = pp.tile([H, N], f32, tag="ysb")      # gather source Y
            xq = pp.tile([H, NQ], bf16, tag="xq")      # per-query X (+b_lm1)
            lf = pp.tile([H, NQ], bf16, tag="lf")      # edge-pooled features
            gmax = pp.tile([H, 2], f32, tag="gmax")
            gsum = pp.tile([H, 2], f32, tag="gsum")
            gstat = pp.tile([H, 4], bf16, tag="gstat")
            gf_bc = pp.tile([H, 2 * QT], bf16, tag="gfbc")  # gf broadcast (2 G-blocks)

            # ---------------- weights / biases ----------------
            def load_w_bf(name, rows, row0=0, cols=None, col0=0, scale=None):
                cols = cols if cols is not None else wd[name].shape[1]
                t_f = pp.tile([rows, cols], f32, tag=f"{name}_{row0}_{col0}_{scale}_f")
                nc.gpsimd.dma_start(t_f[:, :], wd[name][row0:row0 + rows, col0:col0 + cols])
                t_b = pp.tile([rows, cols], bf16, tag=f"{name}_{row0}_{col0}_{scale}_b")
                if scale is None:
                    nc.vector.tensor_copy(t_b[:, :], t_f[:, :])
                else:
                    nc.vector.tensor_scalar_mul(t_b[:, :], t_f[:, :], scale)
                return t_f, t_b

            def load_bias(name, rows, row0=0):
                t = pp.tile([rows, 1], f32, tag=f"{name}_{row0}")
                nc.gpsimd.dma_start(t[:, :], wd[name][row0:row0 + rows].rearrange("o -> o ()"))
                return t

            _, wpe1 = load_w_bf("W_pe1", 6)
            _, wpe2 = load_w_bf("W_pe2", H)
            wl1a_f, _ = load_w_bf("W_lm1", H, 0)
            wl1b_f, wl1b = load_w_bf("W_lm1", H, H)
            _, wl1pm = load_w_bf("W_lm1", 6, 2 * H)
            _, wl1pm_n = load_w_bf("W_lm1", 6, 2 * H, scale=-1.0)
            wdiff = pp.tile([H, H], bf16, tag="wdiff")
            nc.vector.tensor_sub(wdiff[:, :], wl1a_f[:, :], wl1b_f[:, :])
            _, wlm2 = load_w_bf("W_lm2", H)
            wcp = [[load_w_bf("W_cp", H, r * H, H, gb * H)[1] for r in range(2)]
                   for gb in range(2)]
            wgp = [[load_w_bf("W_gp", H, c * H, H, gb * H)[1] for c in range(4)]
                   for gb in range(2)]
            wh1 = {h: [load_w_bf(f"W_{h}1", H, c * H)[1] for c in range(4)]
                   for h in ("cl", "ph", "bh")}
            wh2 = {h: load_w_bf(f"W_{h}2", H)[1] for h in ("cl", "ph", "bh")}

            b_pe1 = load_bias("b_pe1", H)
            b_pe2 = load_bias("b_pe2", H)
            b_lm1 = load_bias("b_lm1", H)
            b_lm2 = load_bias("b_lm2", H)
            b_cp = [load_bias("b_cp", H, gb * H) for gb in range(2)]
            b_gp = [load_bias("b_gp", H, gb * H) for gb in range(2)]
            b_h1 = {h: load_bias(f"b_{h}1", H) for h in ("cl", "ph", "bh")}
            # head-2 biases: added via rank-1 (ones x bias) matmul accumulate
            b_h2row = {}
            for h, od in (("cl", NCLS), ("ph", PD), ("bh", 1)):
                row = pp.tile([1, od], f32, tag=f"b_{h}2_row")
                nc.gpsimd.dma_start(row[:, :], wd[f"b_{h}2"][:].rearrange("o -> () o"))
                rowb = pp.tile([1, od], bf16, tag=f"b_{h}2_rowb")
                nc.vector.tensor_copy(rowb[:, :], row[:, :])
                b_h2row[h] = rowb
            ones1 = pp.tile([1, QT], bf16, tag="ones1")
            nc.vector.memset(ones1[:, :], 1.0)

            ones3 = pp.tile([3, 1], f32, tag="ones3")
            nc.vector.memset(ones3[:, :], 1.0)

            # ---------------- prep: xT, cw, pf, Y, X ----------------
            with tc.tile_pool(name="prep", bufs=2) as prep, \
                 tc.tile_pool(name="prep_ps", bufs=3, space="PSUM") as prep_ps, \
                 tc.tile_pool(name="prep1", bufs=1) as prep1:
                xt6 = prep1.tile([6, N], f32, tag="xt6")
                xb6 = prep1.tile([6, N], bf16, tag="xb6")
                nc.sync.dma_start(xt6[0:3, :], cpts[:, :].rearrange("n c -> c n"))
                nc.sync.dma_start(xt6[3:6, :], cnrm[:, :].rearrange("n c -> c n"))
                nc.vector.tensor_copy(xb6[:, :], xt6[:, :])
                nc.vector.tensor_copy(cw[0:3, :], xt6[0:3, :])
                p2row = prep1.tile([1, N], f32, tag="p2row")
                for c in range(NF):
                    sl = slice(c * FD, (c + 1) * FD)
                    sq = prep.tile([3, FD], f32, tag="sq")
                    nc.vector.tensor_mul(sq[:, :], xt6[0:3, sl], xt6[0:3, sl])
                    ps_p2 = prep_ps.tile([1, FD], f32, tag="pps")
                    nc.tensor.matmul(ps_p2[:, :], ones3[:, :], sq[:, :],
                                     start=True, stop=True)
                    nc.scalar.mul(p2row[0:1, sl], ps_p2[:, :], 0.5)
                nc.sync.dma_start(cw[3:4, :], p2row[:, :])
                for c in range(NF):
                    sl = slice(c * FD, (c + 1) * FD)
                    ps_a = prep_ps.tile([H, FD], f32, tag="pps")
                    nc.tensor.matmul(ps_a[:, :], wpe1[:, :], xb6[:, sl],
                                     start=True, stop=True)
                    hch = prep.tile([H, FD], bf16, tag="hch")
                    nc.scalar.activation(hch[:, :], ps_a[:, :], AF.Relu,
                                         bias=b_pe1[:, 0:1], scale=1.0)
                    ps_b = prep_ps.tile([H, FD], f32, tag="pps")
                    nc.tensor.matmul(ps_b[:, :], wpe2[:, :], hch[:, :],
                                     start=True, stop=True)
                    nc.scalar.activation(pf[:, sl], ps_b[:, :], AF.Relu,
                                         bias=b_pe2[:, 0:1], scale=1.0)
                for c in range(NF):
                    sl = slice(c * FD, (c + 1) * FD)
                    ps_y = prep_ps.tile([H, FD], f32, tag="pps")
                    nc.tensor.matmul(ps_y[:, :], wl1b[:, :], pf[:, sl],
                                     start=True, stop=False)
                    nc.tensor.matmul(ps_y[:, :], wl1pm[:, :], xb6[:, sl],
                                     start=False, stop=True)
                    nc.scalar.copy(ysb[:, sl], ps_y[:, :])
                for c in range(NQ // FD):
                    sl = slice(c * FD, (c + 1) * FD)
                    ps_x = prep_ps.tile([H, FD], f32, tag="pps")
                    nc.tensor.matmul(ps_x[:, :], wdiff[:, :], pf[:, sl],
                                     start=True, stop=False)
                    nc.tensor.matmul(ps_x[:, :], wl1pm_n[:, :], xb6[:, sl],
                                     start=False, stop=True)
                    nc.scalar.activation(xq[:, sl], ps_x[:, :], AF.Identity,
                                         bias=b_lm1[:, 0:1], scale=1.0)

            nc.vector.memset(gmax[:, :], NEG)
            nc.vector.memset(gsum[:, :], 0.0)

            # ---------------- main loop A: kNN + edge MLP + ctx stats ----------
            with tc.tile_pool(name="scores_p", bufs=2) as scores_p, \
                 tc.tile_pool(name="loopa", bufs=2) as la, \
                 tc.tile_pool(name="loopa1", bufs=1) as la1, \
                 tc.tile_pool(name="loopa4", bufs=2) as la4, \
                 tc.tile_pool(name="ps_sc", bufs=4, space="PSUM") as ps_sc, \
                 tc.tile_pool(name="ps_h2", bufs=2, space="PSUM") as ps_h2, \
                 tc.tile_pool(name="ps_cx", bufs=2, space="PSUM") as ps_cx:
                for t in range(NT):
                    tq = slice(t * QT, (t + 1) * QT)
                    # --- scores = <pq,pc> - p2c/2 ---
                    qlt = la.tile([4, QT], f32, tag="qlt")
                    nc.vector.memset(qlt[:, :], -1.0)
                    nc.vector.tensor_copy(qlt[0:3, :], cw[0:3, tq])
                    scores = scores_p.tile([QT, N], f32, tag="scores")
                    if os.environ.get("KVAR", "") == "noscores":
                        nc.vector.memset(scores[:, 0:QT], float(t + 1))
                    else:
                     for c in range(NF):
                        sl = slice(c * FD, (c + 1) * FD)
                        ps = ps_sc.tile([QT, FD], f32, tag="ps_s")
                        nc.tensor.matmul(ps[:, :], qlt[:, :], cw[:, sl],
                                         start=True, stop=True)
                        nc.scalar.copy(scores[:, sl], ps[:, :])
                    VAR = os.environ.get("KVAR", "")
                    # --- top-17: chunk max8 -> 3 extraction rounds ---
                    m_t = la1.tile([QT, NCHK * 8], f32, tag="m_t")
                    if VAR == "noknn":
                        it_ = la.tile([QT, 24], u16, tag="it_")
                        nc.vector.memset(it_[:, :], 0)
                    else:
                     for c in range(NCHK):
                        nc.vector.max(m_t[:, c * 8:(c + 1) * 8],
                                      scores[:, c * CH:(c + 1) * CH])
                    if VAR != "noknn":
                     tt = la.tile([QT, 24], f32, tag="tt")
                     nc.vector.max(tt[:, 0:8], m_t[:, :])
                     nc.vector.match_replace(m_t[:, :], tt[:, 0:8], m_t[:, :], NEG)
                     nc.vector.max(tt[:, 8:16], m_t[:, :])
                     nc.vector.match_replace(m_t[:, :], tt[:, 8:16], m_t[:, :], NEG)
                     nc.vector.max(tt[:, 16:24], m_t[:, :])
                     it_ = la.tile([QT, 24], u16, tag="it_")
                     for r in range(3):
                        nc.vector.max_index(it_[:, r * 8:(r + 1) * 8],
                                            tt[:, r * 8:(r + 1) * 8], scores[:, :])
                    # --- wrap indices for the column gather ---
                    ipad = la.tile([QT, 128], u16, tag="ipad")
                    nc.vector.tensor_copy(ipad[:, 0:16], it_[:, 1:17])
                    nc.vector.tensor_copy(ipad[:, 16:32], it_[:, 1:17])
                    itt = la.tile([128, QT], u16, tag="itt")
                    nc.sync.dma_start_transpose(itt[:, :], ipad[:, :])
                    idxs = la4.tile([H, QT], i16, tag="idxs")
                    for g in range(4):
                        nc.gpsimd.tensor_copy(idxs[g * 32:(g + 1) * 32, :],
                                              itt[0:32, :])
                    yg = la1.tile([H, QT * K], f32, tag="yg")
                    if VAR == "nogather":
                        nc.scalar.copy(yg[:, :], ysb[:, 0:QT * K])
                    else:
                     nc.gpsimd.ap_gather(
                        yg[:, :].rearrange("p m -> p m ()"),
                        ysb[:, :].rearrange("p n -> p n ()"),
                        idxs[:, :], channels=H, num_elems=N, d=1,
                        num_idxs=QT * K)
                    # --- h1 = relu(Y[c] + X[q]) ; h2 ; max over K ---
                    h1 = la.tile([H, QT * K], bf16, tag="h1")
                    nc.vector.tensor_tensor(
                        h1[:, :].rearrange("p (q j) -> p q j", j=K),
                        yg[:, :].rearrange("p (q j) -> p q j", j=K),
                        xq[:, tq].rearrange("p q -> p q ()").broadcast_to([H, QT, K]),
                        op=ALU.add)
                    nc.scalar.activation(h1[:, :], h1[:, :], AF.Relu, bias=0.0,
                                         scale=1.0)
                    lfr = la.tile([H, QT], f32, tag="lfr")
                    for c in range(4):
                        ps2 = ps_h2.tile([H, FD], f32, tag="ps2")
                        nc.tensor.matmul(ps2[:, :], wlm2[:, :],
                                         h1[:, c * FD:(c + 1) * FD],
                                         start=True, stop=True)
                        nc.vector.reduce_max(
                            lfr[:, c * 32:(c + 1) * 32],
                            ps2[:, :].rearrange("p (q j) -> p q j", j=K),
                            axis=mybir.AxisListType.X)
                    nc.scalar.activation(lf[:, tq], lfr[:, :], AF.Relu,
                                         bias=b_lm2[:, 0:1], scale=1.0)
                    # --- ctx + global pooling stats ---
                    for gb in range(2):
                        psc = ps_cx.tile([H, QT], f32, tag="psc")
                        nc.tensor.matmul(psc[:, :], wcp[gb][0][:, :], pf[:, tq],
                                         start=True, stop=False)
                        nc.tensor.matmul(psc[:, :], wcp[gb][1][:, :], lf[:, tq],
                                         start=False, stop=True)
                        cxs = la.tile([H, QT], f32, tag="cxs")
                        nc.scalar.activation(cxs[:, :], psc[:, :], AF.Relu,
                                             bias=b_cp[gb][:, 0:1], scale=1.0)
                        red = la.tile([H, 2], f32, tag="red")
                        nc.vector.reduce_max(red[:, 0:1], cxs[:, :],
                                             axis=mybir.AxisListType.X)
                        nc.vector.reduce_sum(red[:, 1:2], cxs[:, :],
                                             axis=mybir.AxisListType.X)
                        nc.vector.tensor_max(gmax[:, gb:gb + 1], gmax[:, gb:gb + 1],
                                             red[:, 0:1])
                        nc.vector.tensor_add(gsum[:, gb:gb + 1], gsum[:, gb:gb + 1],
                                             red[:, 1:2])

            # ---------------- global pooling collective + gf ----------------
            # All-8 AllReduce over per-cloud slots (cols 2b:2b+2); each core
            # contributes only its own cloud's slot (neutral pads elsewhere).
            cmask_row = pp.tile([1, 2 * B], f32, tag="cmask_row")
            nc.sync.dma_start(cmask_row[:, :], cmask_d[:, :])
            cmask_rowb = pp.tile([1, 2 * B], bf16, tag="cmask_rowb")
            nc.vector.tensor_copy(cmask_rowb[:, :], cmask_row[:, :])
            cmask_bc = pp.tile([H, 2 * B], f32, tag="cmask_bc")
            with tc.tile_pool(name="ps_cm", bufs=1, space="PSUM") as ps_cm:
                ps_c = ps_cm.tile([H, 2 * B], f32, tag="ps_c")
                nc.tensor.matmul(ps_c[:, :], ones1[:, :], cmask_rowb[:, :],
                                 start=True, stop=True)
                nc.scalar.copy(cmask_bc[:, :], ps_c[:, :])
            cpad = pp.tile([H, 2 * B], f32, tag="cpad")
            nc.vector.tensor_scalar(cpad[:, :], cmask_bc[:, :], -NEG, NEG,
                                    op0=ALU.mult, op1=ALU.add)
            gin = pp.tile([H, 2 * B], f32, tag="gin")

            def rep(ap):  # [H,2] -> [H,B,2] broadcast view
                return ap.rearrange("p c -> p () c").broadcast_to([H, B, 2])

            gin_v = gin[:, :].rearrange("p (b c) -> p b c", c=2)
            cm_v = cmask_bc[:, :].rearrange("p (b c) -> p b c", c=2)
            nc.vector.tensor_mul(gin_v, rep(gmax[:, :]), cm_v)
            nc.vector.tensor_add(gin[:, :], gin[:, :], cpad[:, :])
            nc.sync.dma_start(cc_max_in[:, :], gin[:, :])
            nc.gpsimd.collective_compute(
                "AllReduce", ALU.max, replica_groups=GROUPS,
                ins=[cc_max_in[:, :]], outs=[cc_max_out[:, :]])
            nc.vector.tensor_mul(gin_v, rep(gsum[:, :]), cm_v)
            nc.sync.dma_start(cc_sum_in[:, :], gin[:, :])
            nc.gpsimd.collective_compute(
                "AllReduce", ALU.add, replica_groups=GROUPS,
                ins=[cc_sum_in[:, :]], outs=[cc_sum_out[:, :]])
            gmaxr = pp.tile([H, 2 * B], f32, tag="gmaxr")
            gsumr = pp.tile([H, 2 * B], f32, tag="gsumr")
            nc.sync.dma_start(gmaxr[:, :], cc_max_out[:, :])
            nc.sync.dma_start(gsumr[:, :], cc_sum_out[:, :])
            # mask out other clouds (ctx >= 0 so 0-pads are neutral for max too)
            nc.vector.tensor_mul(gmaxr[:, :], gmaxr[:, :], cmask_bc[:, :])
            nc.vector.tensor_mul(gsumr[:, :], gsumr[:, :], cmask_bc[:, :])
            own = pp.tile([H, 4], f32, tag="own")
            nc.vector.reduce_max(
                own[:, 0:2], gmaxr[:, :].rearrange("p (b c) -> p c b", c=2),
                axis=mybir.AxisListType.X)
            nc.vector.reduce_sum(
                own[:, 2:4], gsumr[:, :].rearrange("p (b c) -> p c b", c=2),
                axis=mybir.AxisListType.X)
            nc.vector.tensor_copy(gstat[:, 0:2], own[:, 0:2])
            nc.vector.tensor_scalar_mul(gstat[:, 2:4], own[:, 2:4], 1.0 / N)

            with tc.tile_pool(name="ps_gf", bufs=2, space="PSUM") as ps_gf:
                for gb in range(2):
                    psg = ps_gf.tile([H, 1], f32, tag="psg")
                    for c in range(4):
                        nc.tensor.matmul(psg[:, :], wgp[gb][c][:, :],
                                         gstat[:, c:c + 1],
                                         start=(c == 0), stop=(c == 3))
                    gfv = pp.tile([H, 1], bf16, tag=f"gfv{gb}")
                    nc.scalar.activation(gfv[:, :], psg[:, :], AF.Relu,
                                         bias=b_gp[gb][:, 0:1], scale=1.0)
                    nc.vector.tensor_copy(
                        gf_bc[:, gb * QT:(gb + 1) * QT],
                        gfv[:, 0:1].broadcast_to([H, QT]))

            # ---------------- loop B: heads ----------------
            with tc.tile_pool(name="loopb", bufs=3) as lb, \
                 tc.tile_pool(name="ps_b1", bufs=3, space="PSUM") as ps_b1, \
                 tc.tile_pool(name="ps_b2", bufs=3, space="PSUM") as ps_b2:
                for t in range(NT):
                    tq = slice(t * QT, (t + 1) * QT)
                    osb = lb.tile([QT, NCLS + PD + 1], f32, tag="osb")
                    for h, od, off in (("cl", NCLS, 0), ("ph", PD, NCLS),
                                       ("bh", 1, NCLS + PD)):
                        psh = ps_b1.tile([H, QT], f32, tag="psh")
                        nc.tensor.matmul(psh[:, :], wh1[h][0][:, :], pf[:, tq],
                                         start=True, stop=False)
                        nc.tensor.matmul(psh[:, :], wh1[h][1][:, :], lf[:, tq],
                                         start=False, stop=False)
                        nc.tensor.matmul(psh[:, :], wh1[h][2][:, :],
                                         gf_bc[:, 0:QT], start=False, stop=False)
                        nc.tensor.matmul(psh[:, :], wh1[h][3][:, :],
                                         gf_bc[:, QT:2 * QT], start=False, stop=True)
                        hx = lb.tile([H, QT], bf16, tag=f"hx{h}")
                        nc.scalar.activation(hx[:, :], psh[:, :], AF.Relu,
                                             bias=b_h1[h][:, 0:1], scale=1.0)
                        ps2h = ps_b2.tile([QT, od], f32, tag="ps2h")
                        nc.tensor.matmul(ps2h[:, :], hx[:, :], wh2[h][:, 0:od],
                                         start=True, stop=False)
                        nc.tensor.matmul(ps2h[:, :], ones1[:, :],
                                         b_h2row[h][:, :], start=False, stop=True)
                        nc.scalar.copy(osb[:, off:off + od], ps2h[:, :])
                    nc.sync.dma_start(out_d[t * QT:(t + 1) * QT, :], osb[:, :])

    nc.finalize()
    _legalize_waits(nc)
    return nc


DMA_WAIT_LIMIT_TYPES = ("InstDMACopy",)


def _legalize_waits(nc):
    """TPB instructions hold 1 sync wait (DMA descriptors 2); move overflow
    onto same-engine single-wait Drains inserted just before (engine
    programs execute in-order, so the waits still gate the instruction)."""
    for func in nc.m.functions:
        for block in func.blocks:
            out = []
            for ins in block.instructions:
                si = ins.sync_info
                limit = 1
                if si is not None and len(si.on_wait) > limit:
                    waits = list(si.on_wait)
                    keep = waits[-limit:]
                    for i, w in enumerate(waits[:-limit]):
                        d = mybir.InstDrain(name=f"{ins.name}-w{i}", ins=[],
                                            outs=[], bass_is_fusable=False)
                        d.engine = ins.engine
                        d.sync_info = mybir.SyncInfo(on_wait=[w], on_update=[])
                        out.append(d)
                    ins.sync_info = mybir.SyncInfo(
                        on_wait=keep, on_update=list(si.on_update))
                out.append(ins)
            block.instructions = out


_CACHED = {}


def _get_program():
    if "nc" not in _CACHED:
        _CACHED["nc"] = build_program()
    return _CACHED["nc"]


def run_cores(inputs, trace=False):
    """Build per-core input maps, run on 8 cores, return per-core 'out' arrays."""
    points = np.asarray(inputs["points"], np.float32)
    normals = np.asarray(inputs["normals"], np.float32)
    in_maps = []
    for core in range(8):
        b, half = core // 2, core % 2
        shift = -half * NQ
        cmask = np.zeros((1, 2 * B), np.float32)
        cmask[0, 2 * b:2 * b + 2] = 1.0
        m = {
            "cpts": np.ascontiguousarray(np.roll(points[b], shift, axis=0)),
            "cnrm": np.ascontiguousarray(np.roll(normals[b], shift, axis=0)),
            "cmask": cmask,
        }
        for name, _ in WEIGHT_SPECS:
            m[name] = np.ascontiguousarray(np.asarray(inputs[name], np.float32))
        in_maps.append(m)
    nc = _get_program()
    res = run_bass_kernel_spmd(nc, in_maps, core_ids=list(range(8)), trace=trace)
    return res


def kernel(**inputs):
    assert int(inputs["k"]) == K
    res = run_cores(inputs, trace=False)
    outs = [res.results[c]["out"] for c in range(8)]
    logits = np.zeros((B, N, NCLS), np.float32)
    param = np.zeros((B, N, PD), np.float32)
    boundary = np.zeros((B, N), np.float32)
    for core in range(8):
        b, half = core // 2, core % 2
        o = outs[core].reshape(NQ, NCLS + PD + 1)
        rows = slice(half * NQ, (half + 1) * NQ)
        logits[b, rows] = o[:, :NCLS]
        param[b, rows] = o[:, NCLS:NCLS + PD]
        boundary[b, rows] = o[:, NCLS + PD]
    return logits, param, boundary


def build_trivial_program():
    """Same I/O signature, near-zero compute: for overhead-differencing."""
    nc = bacc_mod.Bacc(num_devices=8)
    nc.declare_dram_parameter("cpts", [N, 3], f32, isOutput=False)
    nc.declare_dram_parameter("cnrm", [N, 3], f32, isOutput=False)
    cmask_d = nc.declare_dram_parameter("cmask", [1, 2 * B], f32, isOutput=False)
    for name, shape in WEIGHT_SPECS:
        nc.declare_dram_parameter(name, list(shape), f32, isOutput=False)
    out_d = nc.declare_dram_parameter("out", [NQ, NCLS + PD + 1], f32,
                                      isOutput=True)
    with TileContext(nc) as tc:
        with tc.tile_pool(name="t", bufs=1) as tp:
            t = tp.tile([1, 2 * B], f32, tag="t")
            nc.sync.dma_start(t[:, :], cmask_d[:, :])
            nc.sync.dma_start(out_d[0:1, 0:2 * B], t[:, :])
    nc.finalize()
    _legalize_waits(nc)
    return nc


def run_cores_nc(nc, inputs, reps=1):
    import time as _t
    points = np.asarray(inputs["points"], np.float32)
    normals = np.asarray(inputs["normals"], np.float32)
    in_maps = []
    for core in range(8):
        b, half = core // 2, core % 2
        cmask = np.zeros((1, 2 * B), np.float32)
        cmask[0, 2 * b:2 * b + 2] = 1.0
        m = {
            "cpts": np.ascontiguousarray(np.roll(points[b], -half * NQ, axis=0)),
            "cnrm": np.ascontiguousarray(np.roll(normals[b], -half * NQ, axis=0)),
            "cmask": cmask,
        }
        for name, _ in WEIGHT_SPECS:
            m[name] = np.ascontiguousarray(np.asarray(inputs[name], np.float32))
        in_maps.append(m)
    times = []
    for _ in range(reps):
        t0 = _t.time()
        res = run_bass_kernel_spmd(nc, in_maps, core_ids=list(range(8)))
        times.append(_t.time() - t0)
    return res, times


# revision 31
# speedup vs baseline: 1.2454x; 1.1292x over previous
"""Trainium2 Bass kernel for nn_LocalPointModel (kNN point-cloud network).

Sharding: 2 cores per cloud (B=4 x 2 halves = 8 cores). Each core computes
4096 query rows of its cloud. The per-core input cloud is ROTATED on the host
so that a core's queries are always rows 0:4096 (fully uniform SPMD kernel,
no dynamic addressing). kNN / gather / edge-MLP / heads are per-core local;
only the global ctx pooling (max+mean over the full cloud) needs a 2-core
AllReduce per cloud pair.

kNN: score[q,c] = <pq,pc> - |pc|^2/2  (= -d2/2 + const(q), order-preserving)
computed by a contract-dim-4 fp32 matmul; top-17 per row via 3 full-row DVE
rounds of (max8 -> max_index -> match_replace[-inf]) — ucode-dispatched ops,
so op COUNT is minimized, not elements. Row max is always self (score diff =
-d2/2 <= 0), so neighbors = extracted[1:17]. Exact (no chunk approximation).

Edge MLP layer 1 is decomposed as  edge@W1 = X[q] + Y[c]  with
  Y = pf@Wn + pts@Wp + nrm@Wm     (gather source, per cloud point)
  X = pf@(Wc-Wn) - pts@Wp - nrm@Wm + b1   (per query)
so the gather feeds a single elementwise add instead of a 262x128 matmul
per edge. Neighbor gather = gpsimd.ap_gather on columns of Y [H=128part, N].
"""

import os
import numpy as np

import concourse.bass as bass
import concourse.bacc as bacc_mod
import concourse.mybir as mybir
from concourse.tile import TileContext
from concourse.bass_utils import run_bass_kernel_spmd

B, N, K, H, G, NCLS, PD = 4, 8192, 16, 128, 256, 32, 16
NQ = N // 2            # queries per core
QT = 128               # queries per tile
NT = NQ // QT          # 32 query tiles
FD = 512               # matmul moving-dim chunk
NF = N // FD           # 16
CH = 512               # top-k phase-1 chunk width
NCHK = N // CH         # 32 chunks
NEG = -3.0e38

f32 = mybir.dt.float32
bf16 = mybir.dt.bfloat16
u16 = mybir.dt.uint16
i16 = mybir.dt.int16
AF = mybir.ActivationFunctionType
ALU = mybir.AluOpType

# (name, shape) of every weight input, in reference order
WEIGHT_SPECS = [
    ("W_pe1", (6, H)), ("b_pe1", (H,)), ("W_pe2", (H, H)), ("b_pe2", (H,)),
    ("W_lm1", (2 * H + 6, H)), ("b_lm1", (H,)), ("W_lm2", (H, H)), ("b_lm2", (H,)),
    ("W_cp", (2 * H, G)), ("b_cp", (G,)), ("W_gp", (2 * G, G)), ("b_gp", (G,)),
    ("W_cl1", (2 * H + G, H)), ("b_cl1", (H,)), ("W_cl2", (H, NCLS)), ("b_cl2", (NCLS,)),
    ("W_ph1", (2 * H + G, H)), ("b_ph1", (H,)), ("W_ph2", (H, PD)), ("b_ph2", (PD,)),
    ("W_bh1", (2 * H + G, H)), ("b_bh1", (H,)), ("W_bh2", (H, 1)), ("b_bh2", (1,)),
]


def build_program():
    nc = bacc_mod.Bacc(num_devices=8)

    cpts = nc.declare_dram_parameter("cpts", [N, 3], f32, isOutput=False)
    cnrm = nc.declare_dram_parameter("cnrm", [N, 3], f32, isOutput=False)
    cmask_d = nc.declare_dram_parameter("cmask", [1, 2 * B], f32, isOutput=False)
    wd = {
        name: nc.declare_dram_parameter(name, list(shape), f32, isOutput=False)
        for name, shape in WEIGHT_SPECS
    }
    out_d = nc.declare_dram_parameter("out", [NQ, NCLS + PD + 1], f32, isOutput=True)

    # collective scratch: per-cloud slots, all-8 AllReduce (pads are neutral)
    cc_max_in = nc.dram_tensor("cc_max_in", [H, 2 * B], f32)
    cc_max_out = nc.dram_tensor("cc_max_out", [H, 2 * B], f32, addr_space="Shared")
    cc_sum_in = nc.dram_tensor("cc_sum_in", [H, 2 * B], f32)
    cc_sum_out = nc.dram_tensor("cc_sum_out", [H, 2 * B], f32, addr_space="Shared")
    GROUPS = [[0, 1, 2, 3, 4, 5, 6, 7]]

    with TileContext(nc) as tc:
        with tc.tile_pool(name="persist", bufs=1) as pp:
            # ---------------- persistent tiles ----------------
            cw = pp.tile([4, N], f32, tag="cw")        # score rhs: [px,py,pz,p2/2]
            pf = pp.tile([H, N], bf16, tag="pf")       # point features, full cloud
            ysb = pp.tile([H, N], f32, tag="ysb")      # gather source Y
            xq = pp.tile([H, NQ], bf16, tag="xq")      # per-query X (+b_lm1)
            lf = pp.tile([H, NQ], bf16, tag="lf")      # edge-pooled features
            gmax = pp.tile([H, 2], f32, tag="gmax")
            gsum = pp.tile([H, 2], f32, tag="gsum")
            gstat = pp.tile([H, 4], bf16, tag="gstat")
            gf_bc = pp.tile([H, 2 * QT], bf16, tag="gfbc")  # gf broadcast (2 G-blocks)

            # ---------------- weights / biases ----------------
            def load_w_bf(name, rows, row0=0, cols=None, col0=0, scale=None):
                cols = cols if cols is not None else wd[name].shape[1]
                t_f = pp.tile([rows, cols], f32, tag=f"{name}_{row0}_{col0}_{scale}_f")
                nc.gpsimd.dma_start(t_f[:, :], wd[name][row0:row0 + rows, col0:col0 + cols])
                t_b = pp.tile([rows, cols], bf16, tag=f"{name}_{row0}_{col0}_{scale}_b")
                if scale is None:
                    nc.vector.tensor_copy(t_b[:, :], t_f[:, :])
                else:
                    nc.vector.tensor_scalar_mul(t_b[:, :], t_f[:, :], scale)
                return t_f, t_b

            def load_bias(name, rows, row0=0):
                t = pp.tile([rows, 1], f32, tag=f"{name}_{row0}")
                nc.gpsimd.dma_start(t[:, :], wd[name][row0:row0 + rows].rearrange("o -> o ()"))
                return t

            _, wpe1 = load_w_bf("W_pe1", 6)
            _, wpe2 = load_w_bf("W_pe2", H)
            wl1a_f, _ = load_w_bf("W_lm1", H, 0)
            wl1b_f, wl1b = load_w_bf("W_lm1", H, H)
            _, wl1pm = load_w_bf("W_lm1", 6, 2 * H)
            _, wl1pm_n = load_w_bf("W_lm1", 6, 2 * H, scale=-1.0)
            wdiff = pp.tile([H, H], bf16, tag="wdiff")
            nc.vector.tensor_sub(wdiff[:, :], wl1a_f[:, :], wl1b_f[:, :])
            _, wlm2 = load_w_bf("W_lm2", H)
            wcp = [[load_w_bf("W_cp", H, r * H, H, gb * H)[1] for r in range(2)]
                   for gb in range(2)]
            wgp = [[load_w_bf("W_gp", H, c * H, H, gb * H)[1] for c in range(4)]
                   for gb in range(2)]
            wh1 = {h: [load_w_bf(f"W_{h}1", H, c * H)[1] for c in range(4)]
                   for h in ("cl", "ph", "bh")}
            wh2 = {h: load_w_bf(f"W_{h}2", H)[1] for h in ("cl", "ph", "bh")}

            b_pe1 = load_bias("b_pe1", H)
            b_pe2 = load_bias("b_pe2", H)
            b_lm1 = load_bias("b_lm1", H)
            b_lm2 = load_bias("b_lm2", H)
            b_cp = [load_bias("b_cp", H, gb * H) for gb in range(2)]
            b_gp = [load_bias("b_gp", H, gb * H) for gb in range(2)]
            b_h1 = {h: load_bias(f"b_{h}1", H) for h in ("cl", "ph", "bh")}
            # head-2 biases: added via rank-1 (ones x bias) matmul accumulate
            b_h2row = {}
            for h, od in (("cl", NCLS), ("ph", PD), ("bh", 1)):
                row = pp.tile([1, od], f32, tag=f"b_{h}2_row")
                nc.gpsimd.dma_start(row[:, :], wd[f"b_{h}2"][:].rearrange("o -> () o"))
                rowb = pp.tile([1, od], bf16, tag=f"b_{h}2_rowb")
                nc.vector.tensor_copy(rowb[:, :], row[:, :])
                b_h2row[h] = rowb
            ones1 = pp.tile([1, QT], bf16, tag="ones1")
            nc.vector.memset(ones1[:, :], 1.0)

            ones3 = pp.tile([3, 1], f32, tag="ones3")
            nc.vector.memset(ones3[:, :], 1.0)

            # ---------------- prep: xT, cw, pf, Y, X ----------------
            with tc.tile_pool(name="prep", bufs=2) as prep, \
                 tc.tile_pool(name="prep_ps", bufs=3, space="PSUM") as prep_ps, \
                 tc.tile_pool(name="prep1", bufs=1) as prep1:
                xt6 = prep1.tile([6, N], f32, tag="xt6")
                xb6 = prep1.tile([6, N], bf16, tag="xb6")
                nc.sync.dma_start(xt6[0:3, :], cpts[:, :].rearrange("n c -> c n"))
                nc.sync.dma_start(xt6[3:6, :], cnrm[:, :].rearrange("n c -> c n"))
                nc.vector.tensor_copy(xb6[:, :], xt6[:, :])
                nc.vector.tensor_copy(cw[0:3, :], xt6[0:3, :])
                p2row = prep1.tile([1, N], f32, tag="p2row")
                for c in range(NF):
                    sl = slice(c * FD, (c + 1) * FD)
                    sq = prep.tile([3, FD], f32, tag="sq")
                    nc.vector.tensor_mul(sq[:, :], xt6[0:3, sl], xt6[0:3, sl])
                    ps_p2 = prep_ps.tile([1, FD], f32, tag="pps")
                    nc.tensor.matmul(ps_p2[:, :], ones3[:, :], sq[:, :],
                                     start=True, stop=True)
                    nc.scalar.mul(p2row[0:1, sl], ps_p2[:, :], 0.5)
                nc.sync.dma_start(cw[3:4, :], p2row[:, :])
                for c in range(NF):
                    sl = slice(c * FD, (c + 1) * FD)
                    ps_a = prep_ps.tile([H, FD], f32, tag="pps")
                    nc.tensor.matmul(ps_a[:, :], wpe1[:, :], xb6[:, sl],
                                     start=True, stop=True)
                    hch = prep.tile([H, FD], bf16, tag="hch")
                    nc.scalar.activation(hch[:, :], ps_a[:, :], AF.Relu,
                                         bias=b_pe1[:, 0:1], scale=1.0)
                    ps_b = prep_ps.tile([H, FD], f32, tag="pps")
                    nc.tensor.matmul(ps_b[:, :], wpe2[:, :], hch[:, :],
                                     start=True, stop=True)
                    nc.scalar.activation(pf[:, sl], ps_b[:, :], AF.Relu,
                                         bias=b_pe2[:, 0:1], scale=1.0)
                for c in range(NF):
                    sl = slice(c * FD, (c + 1) * FD)
                    ps_y = prep_ps.tile([H, FD], f32, tag="pps")
                    nc.tensor.matmul(ps_y[:, :], wl1b[:, :], pf[:, sl],
                                     start=True, stop=False)
                    nc.tensor.matmul(ps_y[:, :], wl1pm[:, :], xb6[:, sl],
                                     start=False, stop=True)
                    nc.scalar.copy(ysb[:, sl], ps_y[:, :])
                for c in range(NQ // FD):
                    sl = slice(c * FD, (c + 1) * FD)
                    ps_x = prep_ps.tile([H, FD], f32, tag="pps")
                    nc.tensor.matmul(ps_x[:, :], wdiff[:, :], pf[:, sl],
                                     start=True, stop=False)
                    nc.tensor.matmul(ps_x[:, :], wl1pm_n[:, :], xb6[:, sl],
                                     start=False, stop=True)
                    nc.scalar.activation(xq[:, sl], ps_x[:, :], AF.Identity,
                                         bias=b_lm1[:, 0:1], scale=1.0)

            nc.vector.memset(gmax[:, :], NEG)
            nc.vector.memset(gsum[:, :], 0.0)

            # ---------------- main loop A: kNN + edge MLP + ctx stats ----------
            with tc.tile_pool(name="scores_p", bufs=2) as scores_p, \
                 tc.tile_pool(name="loopa", bufs=2) as la, \
                 tc.tile_pool(name="loopa1", bufs=1) as la1, \
                 tc.tile_pool(name="loopa4", bufs=2) as la4, \
                 tc.tile_pool(name="ps_sc", bufs=4, space="PSUM") as ps_sc, \
                 tc.tile_pool(name="ps_h2", bufs=2, space="PSUM") as ps_h2, \
                 tc.tile_pool(name="ps_cx", bufs=2, space="PSUM") as ps_cx:
                for t in range(NT):
                    tq = slice(t * QT, (t + 1) * QT)
                    # --- scores = <pq,pc> - p2c/2 ---
                    qlt = la.tile([4, QT], f32, tag="qlt")
                    nc.vector.memset(qlt[:, :], -1.0)
                    nc.vector.tensor_copy(qlt[0:3, :], cw[0:3, tq])
                    scores = scores_p.tile([QT, N], f32, tag="scores")
                    for c in range(NF):
                        sl = slice(c * FD, (c + 1) * FD)
                        ps = ps_sc.tile([QT, FD], f32, tag="ps_s")
                        nc.tensor.matmul(ps[:, :], qlt[:, :], cw[:, sl],
                                         start=True, stop=True)
                        nc.scalar.copy(scores[:, sl], ps[:, :])
                    # --- top-17 directly on the full row (ucode ops are
                    # dispatch-dominated ~200us each, so minimize op count).
                    # max_index must precede match_replace (which zaps the
                    # found values to -inf for the next round).
                    tt = la.tile([QT, 24], f32, tag="tt")
                    it_ = la.tile([QT, 24], u16, tag="it_")
                    for r in range(3):
                        s8 = slice(r * 8, (r + 1) * 8)
                        nc.vector.max(tt[:, s8], scores[:, :])
                        nc.vector.max_index(it_[:, s8], tt[:, s8], scores[:, :])
                        if r < 2:
                            nc.vector.match_replace(scores[:, :], tt[:, s8],
                                                    scores[:, :], NEG)
                    ipad = la.tile([QT, 128], u16, tag="ipad")
                    nc.vector.tensor_copy(ipad[:, 0:16], it_[:, 1:17])
                    nc.vector.tensor_copy(ipad[:, 16:32], it_[:, 1:17])
                    itt = la.tile([128, QT], u16, tag="itt")
                    nc.sync.dma_start_transpose(itt[:, :], ipad[:, :])
                    idxs = la4.tile([H, QT], i16, tag="idxs")
                    for g in range(4):
                        nc.gpsimd.tensor_copy(idxs[g * 32:(g + 1) * 32, :],
                                              itt[0:32, :])
                    yg = la1.tile([H, QT * K], f32, tag="yg")
                    nc.gpsimd.ap_gather(
                        yg[:, :].rearrange("p m -> p m ()"),
                        ysb[:, :].rearrange("p n -> p n ()"),
                        idxs[:, :], channels=H, num_elems=N, d=1,
                        num_idxs=QT * K)
                    # --- h1 = relu(Y[c] + X[q]) ; h2 ; max over K ---
                    h1 = la.tile([H, QT * K], bf16, tag="h1")
                    nc.vector.tensor_tensor(
                        h1[:, :].rearrange("p (q j) -> p q j", j=K),
                        yg[:, :].rearrange("p (q j) -> p q j", j=K),
                        xq[:, tq].rearrange("p q -> p q ()").broadcast_to([H, QT, K]),
                        op=ALU.add)
                    nc.scalar.activation(h1[:, :], h1[:, :], AF.Relu, bias=0.0,
                                         scale=1.0)
                    lfr = la.tile([H, QT], f32, tag="lfr")
                    for c in range(4):
                        ps2 = ps_h2.tile([H, FD], f32, tag="ps2")
                        nc.tensor.matmul(ps2[:, :], wlm2[:, :],
                                         h1[:, c * FD:(c + 1) * FD],
                                         start=True, stop=True)
                        nc.vector.reduce_max(
                            lfr[:, c * 32:(c + 1) * 32],
                            ps2[:, :].rearrange("p (q j) -> p q j", j=K),
                            axis=mybir.AxisListType.X)
                    nc.scalar.activation(lf[:, tq], lfr[:, :], AF.Relu,
                                         bias=b_lm2[:, 0:1], scale=1.0)
                    # --- ctx + global pooling stats ---
                    for gb in range(2):
                        psc = ps_cx.tile([H, QT], f32, tag="psc")
                        nc.tensor.matmul(psc[:, :], wcp[gb][0][:, :], pf[:, tq],
                                         start=True, stop=False)
                        nc.tensor.matmul(psc[:, :], wcp[gb][1][:, :], lf[:, tq],
                                         start=False, stop=True)
                        cxs = la.tile([H, QT], f32, tag="cxs")
                        nc.scalar.activation(cxs[:, :], psc[:, :], AF.Relu,
                                             bias=b_cp[gb][:, 0:1], scale=1.0)
                        red = la.tile([H, 2], f32, tag="red")
                        nc.vector.reduce_max(red[:, 0:1], cxs[:, :],
                                             axis=mybir.AxisListType.X)
                        nc.vector.reduce_sum(red[:, 1:2], cxs[:, :],
                                             axis=mybir.AxisListType.X)
                        nc.vector.tensor_max(gmax[:, gb:gb + 1], gmax[:, gb:gb + 1],
                                             red[:, 0:1])
                        nc.vector.tensor_add(gsum[:, gb:gb + 1], gsum[:, gb:gb + 1],
                                             red[:, 1:2])

            # ---------------- global pooling collective + gf ----------------
            # All-8 AllReduce over per-cloud slots (cols 2b:2b+2); each core
            # contributes only its own cloud's slot (neutral pads elsewhere).
            cmask_row = pp.tile([1, 2 * B], f32, tag="cmask_row")
            nc.sync.dma_start(cmask_row[:, :], cmask_d[:, :])
            cmask_rowb = pp.tile([1, 2 * B], bf16, tag="cmask_rowb")
            nc.vector.tensor_copy(cmask_rowb[:, :], cmask_row[:, :])
            cmask_bc = pp.tile([H, 2 * B], f32, tag="cmask_bc")
            with tc.tile_pool(name="ps_cm", bufs=1, space="PSUM") as ps_cm:
                ps_c = ps_cm.tile([H, 2 * B], f32, tag="ps_c")
                nc.tensor.matmul(ps_c[:, :], ones1[:, :], cmask_rowb[:, :],
                                 start=True, stop=True)
                nc.scalar.copy(cmask_bc[:, :], ps_c[:, :])
            cpad = pp.tile([H, 2 * B], f32, tag="cpad")
            nc.vector.tensor_scalar(cpad[:, :], cmask_bc[:, :], -NEG, NEG,
                                    op0=ALU.mult, op1=ALU.add)
            gin = pp.tile([H, 2 * B], f32, tag="gin")

            def rep(ap):  # [H,2] -> [H,B,2] broadcast view
                return ap.rearrange("p c -> p () c").broadcast_to([H, B, 2])

            gin_v = gin[:, :].rearrange("p (b c) -> p b c", c=2)
            cm_v = cmask_bc[:, :].rearrange("p (b c) -> p b c", c=2)
            nc.vector.tensor_mul(gin_v, rep(gmax[:, :]), cm_v)
            nc.vector.tensor_add(gin[:, :], gin[:, :], cpad[:, :])
            nc.sync.dma_start(cc_max_in[:, :], gin[:, :])
            nc.gpsimd.collective_compute(
                "AllReduce", ALU.max, replica_groups=GROUPS,
                ins=[cc_max_in[:, :]], outs=[cc_max_out[:, :]])
            nc.vector.tensor_mul(gin_v, rep(gsum[:, :]), cm_v)
            nc.sync.dma_start(cc_sum_in[:, :], gin[:, :])
            nc.gpsimd.collective_compute(
                "AllReduce", ALU.add, replica_groups=GROUPS,
                ins=[cc_sum_in[:, :]], outs=[cc_sum_out[:, :]])
            gmaxr = pp.tile([H, 2 * B], f32, tag="gmaxr")
            gsumr = pp.tile([H, 2 * B], f32, tag="gsumr")
            nc.sync.dma_start(gmaxr[:, :], cc_max_out[:, :])
            nc.sync.dma_start(gsumr[:, :], cc_sum_out[:, :])
            # mask out other clouds (ctx >= 0 so 0-pads are neutral for max too)
            nc.vector.tensor_mul(gmaxr[:, :], gmaxr[:, :], cmask_bc[:, :])
            nc.vector.tensor_mul(gsumr[:, :], gsumr[:, :], cmask_bc[:, :])
            own = pp.tile([H, 4], f32, tag="own")
            nc.vector.reduce_max(
                own[:, 0:2], gmaxr[:, :].rearrange("p (b c) -> p c b", c=2),
                axis=mybir.AxisListType.X)
            nc.vector.reduce_sum(
                own[:, 2:4], gsumr[:, :].rearrange("p (b c) -> p c b", c=2),
                axis=mybir.AxisListType.X)
            nc.vector.tensor_copy(gstat[:, 0:2], own[:, 0:2])
            nc.vector.tensor_scalar_mul(gstat[:, 2:4], own[:, 2:4], 1.0 / N)

            with tc.tile_pool(name="ps_gf", bufs=2, space="PSUM") as ps_gf:
                for gb in range(2):
                    psg = ps_gf.tile([H, 1], f32, tag="psg")
                    for c in range(4):
                        nc.tensor.matmul(psg[:, :], wgp[gb][c][:, :],
                                         gstat[:, c:c + 1],
                                         start=(c == 0), stop=(c == 3))
                    gfv = pp.tile([H, 1], bf16, tag=f"gfv{gb}")
                    nc.scalar.activation(gfv[:, :], psg[:, :], AF.Relu,
                                         bias=b_gp[gb][:, 0:1], scale=1.0)
                    nc.vector.tensor_copy(
                        gf_bc[:, gb * QT:(gb + 1) * QT],
                        gfv[:, 0:1].broadcast_to([H, QT]))

            # ---------------- loop B: heads ----------------
            with tc.tile_pool(name="loopb", bufs=3) as lb, \
                 tc.tile_pool(name="ps_b1", bufs=3, space="PSUM") as ps_b1, \
                 tc.tile_pool(name="ps_b2", bufs=3, space="PSUM") as ps_b2:
                for t in range(NT):
                    tq = slice(t * QT, (t + 1) * QT)
                    osb = lb.tile([QT, NCLS + PD + 1], f32, tag="osb")
                    for h, od, off in (("cl", NCLS, 0), ("ph", PD, NCLS),
                                       ("bh", 1, NCLS + PD)):
                        psh = ps_b1.tile([H, QT], f32, tag="psh")
                        nc.tensor.matmul(psh[:, :], wh1[h][0][:, :], pf[:, tq],
                                         start=True, stop=False)
                        nc.tensor.matmul(psh[:, :], wh1[h][1][:, :], lf[:, tq],
                                         start=False, stop=False)
                        nc.tensor.matmul(psh[:, :], wh1[h][2][:, :],
                                         gf_bc[:, 0:QT], start=False, stop=False)
                        nc.tensor.matmul(psh[:, :], wh1[h][3][:, :],
                                         gf_bc[:, QT:2 * QT], start=False, stop=True)
                        hx = lb.tile([H, QT], bf16, tag=f"hx{h}")
                        nc.scalar.activation(hx[:, :], psh[:, :], AF.Relu,
                                             bias=b_h1[h][:, 0:1], scale=1.0)
                        ps2h = ps_b2.tile([QT, od], f32, tag="ps2h")
                        nc.tensor.matmul(ps2h[:, :], hx[:, :], wh2[h][:, 0:od],
                                         start=True, stop=False)
                        nc.tensor.matmul(ps2h[:, :], ones1[:, :],
                                         b_h2row[h][:, :], start=False, stop=True)
                        nc.scalar.copy(osb[:, off:off + od], ps2h[:, :])
                    nc.sync.dma_start(out_d[t * QT:(t + 1) * QT, :], osb[:, :])

    nc.finalize()
    _legalize_waits(nc)
    return nc


DMA_WAIT_LIMIT_TYPES = ("InstDMACopy",)


def _legalize_waits(nc):
    """TPB instructions hold 1 sync wait (DMA descriptors 2); move overflow
    onto same-engine single-wait Drains inserted just before (engine
    programs execute in-order, so the waits still gate the instruction)."""
    for func in nc.m.functions:
        for block in func.blocks:
            out = []
            for ins in block.instructions:
                si = ins.sync_info
                limit = 1
                if si is not None and len(si.on_wait) > limit:
                    waits = list(si.on_wait)
                    keep = waits[-limit:]
                    for i, w in enumerate(waits[:-limit]):
                        d = mybir.InstDrain(name=f"{ins.name}-w{i}", ins=[],
                                            outs=[], bass_is_fusable=False)
                        d.engine = ins.engine
                        d.sync_info = mybir.SyncInfo(on_wait=[w], on_update=[])
                        out.append(d)
                    ins.sync_info = mybir.SyncInfo(
                        on_wait=keep, on_update=list(si.on_update))
                out.append(ins)
            block.instructions = out


_CACHED = {}


def _get_program():
    if "nc" not in _CACHED:
        _CACHED["nc"] = build_program()
    return _CACHED["nc"]


def run_cores(inputs, trace=False):
    """Build per-core input maps, run on 8 cores, return per-core 'out' arrays."""
    points = np.asarray(inputs["points"], np.float32)
    normals = np.asarray(inputs["normals"], np.float32)
    in_maps = []
    for core in range(8):
        b, half = core // 2, core % 2
        shift = -half * NQ
        cmask = np.zeros((1, 2 * B), np.float32)
        cmask[0, 2 * b:2 * b + 2] = 1.0
        m = {
            "cpts": np.ascontiguousarray(np.roll(points[b], shift, axis=0)),
            "cnrm": np.ascontiguousarray(np.roll(normals[b], shift, axis=0)),
            "cmask": cmask,
        }
        for name, _ in WEIGHT_SPECS:
            m[name] = np.ascontiguousarray(np.asarray(inputs[name], np.float32))
        in_maps.append(m)
    nc = _get_program()
    res = run_bass_kernel_spmd(nc, in_maps, core_ids=list(range(8)), trace=trace)
    return res


def kernel(**inputs):
    assert int(inputs["k"]) == K
    res = run_cores(inputs, trace=False)
    outs = [res.results[c]["out"] for c in range(8)]
    logits = np.zeros((B, N, NCLS), np.float32)
    param = np.zeros((B, N, PD), np.float32)
    boundary = np.zeros((B, N), np.float32)
    for core in range(8):
        b, half = core // 2, core % 2
        o = outs[core].reshape(NQ, NCLS + PD + 1)
        rows = slice(half * NQ, (half + 1) * NQ)
        logits[b, rows] = o[:, :NCLS]
        param[b, rows] = o[:, NCLS:NCLS + PD]
        boundary[b, rows] = o[:, NCLS + PD]
    return logits, param, boundary


def build_trivial_program():
    """Same I/O signature, near-zero compute: for overhead-differencing."""
    nc = bacc_mod.Bacc(num_devices=8)
    nc.declare_dram_parameter("cpts", [N, 3], f32, isOutput=False)
    nc.declare_dram_parameter("cnrm", [N, 3], f32, isOutput=False)
    cmask_d = nc.declare_dram_parameter("cmask", [1, 2 * B], f32, isOutput=False)
    for name, shape in WEIGHT_SPECS:
        nc.declare_dram_parameter(name, list(shape), f32, isOutput=False)
    out_d = nc.declare_dram_parameter("out", [NQ, NCLS + PD + 1], f32,
                                      isOutput=True)
    with TileContext(nc) as tc:
        with tc.tile_pool(name="t", bufs=1) as tp:
            t = tp.tile([1, 2 * B], f32, tag="t")
            nc.sync.dma_start(t[:, :], cmask_d[:, :])
            nc.sync.dma_start(out_d[0:1, 0:2 * B], t[:, :])
    nc.finalize()
    _legalize_waits(nc)
    return nc


def run_cores_nc(nc, inputs, reps=1):
    import time as _t
    points = np.asarray(inputs["points"], np.float32)
    normals = np.asarray(inputs["normals"], np.float32)
    in_maps = []
    for core in range(8):
        b, half = core // 2, core % 2
        cmask = np.zeros((1, 2 * B), np.float32)
        cmask[0, 2 * b:2 * b + 2] = 1.0
        m = {
            "cpts": np.ascontiguousarray(np.roll(points[b], -half * NQ, axis=0)),
            "cnrm": np.ascontiguousarray(np.roll(normals[b], -half * NQ, axis=0)),
            "cmask": cmask,
        }
        for name, _ in WEIGHT_SPECS:
            m[name] = np.ascontiguousarray(np.asarray(inputs[name], np.float32))
        in_maps.append(m)
    times = []
    for _ in range(reps):
        t0 = _t.time()
        res = run_bass_kernel_spmd(nc, in_maps, core_ids=list(range(8)))
        times.append(_t.time() - t0)
    return res, times
